# revision 30
# baseline (speedup 1.0000x reference)
"""Trainium2 Bass kernel for MessagePassingWithPhase.

Reference computation (B=2, N=512, D=128, O=4):
    recv = X @ W1r ; send = X @ W1s
    hidden[b,i,j,:]  = relu(recv[b,i] + send[b,j] + b1)
    messages         = hidden @ W2 + b2
    gate             = sigmoid(cos(phi_i - phi_j) @ Wg + bg)
    agg[b,i]         = sum_j mask[i,j] * (messages * gate)[b,i,j] / cnt_i
    out              = X + (relu(X@Wu1x + agg@Wu1a + bu1) @ Wu2 + bu2)

Mapping (8 cores, receiver axis sharded: 64 receivers/core, both batches):
  * Everything feature-major: (D=128 partitions, node index on the free axis),
    so biases/scales are per-partition columns and no on-device transposes
    exist anywhere.
  * cos(phi_i-phi_j) = cos(phi_i)cos(phi_j) + sin(phi_i)sin(phi_j): the gate
    linear becomes a K=9 bf16 matmul  Wg3.T @ r9  with
      r9[0:8,j] = P8[o,j] * P8[o,i]  (P8 = [cos(phi); sin(phi)], host-computed)
      r9[8,j]   = -50 * (1 - mask[i,j]),  Wg3 = [[Wg;Wg]; ones]
    so the sigmoid saturates to ~0 on non-neighbors: the masked mean becomes
    a plain sum over j scaled by 1/cnt_i (folded in at the end).  The mask
    rows are DMA'd straight into r9 row 8 (one DMA per 16 receivers, shared
    by both batches).
  * Per receiver i: r9 rows 0-7 are one tensor_scalar (scalar = p8r column,
    engine round-robin DVE/ACT); H = relu(send + recv_i + b1) is ONE fused op
    (ACT activation with bias column, or DVE tensor_scalar add+max); message
    matmul W2.T @ H and gate matmul Wg3.T @ r9 go to PSUM (bf16 weights);
    sigmoid(+bg) batches 2 receivers per ACT op; (M + b2) * G runs in-place
    on the message PSUM bank with the fused per-partition accumulator
    (accum_out) producing the neighbor sum directly - no separate reduce.
  * PSUM: 2-receiver groups, (D,1024) message + (D,1024) gate tiles, 4-buffer
    rotation = all 8 banks, so matmuls for group g+1 overlap the sigmoid/
    combine of group g.  Work is spread across PE / ACT / DVE by the
    RELU_ENG / R9_ENG knobs; gpsimd is avoided for sub-128-partition ops
    (an 8-partition op pins one of its 8 Q7 cores: ~16x below line rate).
  * Update MLP + residual run for both batches in one (128, 128) tile set;
    the host re-transposes the (B, D, 64) per-core outputs.
Heavy elementwise/matmul traffic is bf16 (rel err ~3e-5, tolerance 2e-2);
accumulation (PSUM, neighbor sums, epilogue) stays fp32.
REPEAT>1 wraps the body in a hardware loop (tc.For_i), so the program size
stays constant and (wall(R=hi)-wall(R=1))/(hi-1) isolates device time.
"""
import contextlib
import os
import sys
import numpy as np

for _p in ("/opt/trn_rl_repo", "/root/.axon_site/_ro/trn_rl_repo"):
    if os.path.isdir(_p) and _p not in sys.path:
        sys.path.append(_p)

B, N, D, O = 2, 512, 128, 4
NCORES = 8
NPC = N // NCORES  # receivers per core
MASK_NEG = -50.0
QUAD = 16          # receivers per batched build/reduce

# ---- tuning knobs -----------------------------------------------------------
H_DT = "bfloat16"     # dtype of H (relu build out / message-matmul moving+wts)
R9_DT = "bfloat16"    # dtype of gate-rhs rows / gate-matmul moving+wts
S_DT = "bfloat16"     # dtype of the sendT tile feeding the H build
G_DT = "bfloat16"     # dtype of the sigmoid gate tile
REPEAT = 1            # timing aid: run the compute body N times on device
GRP = 2               # receivers per PSUM group (GRP*2 banks * 2 bufs <= 8)
COMBINE = "sttacc"    # "sttacc": fused accum per receiver; "reduce16": v1-style
# per-16-receiver engine assignment for the fused H=relu(send+recv+b1) build
RELU_ENG = ("act",) * 12 + ("dve",) * 4
# engine for the gate-rhs phase-product rows (NEVER "pool": an 8-partition op
# occupies one of gpsimd's 8 Q7 cores and runs ~16x slow)
R9_ENG = ("dve",) * 8 + ("act",) * 8

_CACHE = {}


def _build_program():
    import concourse.bacc as bacc
    import concourse.mybir as mybir
    import concourse.tile as tile

    f32 = mybir.dt.float32
    hdt = getattr(mybir.dt, H_DT)
    rdt = getattr(mybir.dt, R9_DT)
    sdt = getattr(mybir.dt, S_DT)
    gdt = getattr(mybir.dt, G_DT)
    A = mybir.AluOpType
    AF = mybir.ActivationFunctionType

    nc = bacc.Bacc("TRN2", debug=False)

    def din(name, shape, dt=f32):
        return nc.declare_dram_parameter(name, list(shape), dt, isOutput=False)

    xt = din("xt", (B, D, N))          # node features, transposed
    xtr = din("xtr", (D, B * NPC))     # receiver cols of xt, both batches
    p8 = din("p8", (B, 2 * O, N), rdt)    # [cos(phi); sin(phi)] senders
    p8r = din("p8r", (B, 2 * O, NPC))  # receiver cols of p8 (f32: TS scalar)
    # -50*(1-mask) oct rows, in the gate-matmul moving dtype (values exact)
    mneg = din("mneg", (NPC // QUAD, QUAD * N), rdt)
    # all small f32 constants in one blob:
    # [w1r|w1s|w2|wu1x|wu1a|wu2|cinv(both b)|b1c|bgc|b2c|bu1c|bu2c]
    NBLOB = 6 * D + B * NPC + 5
    blob = din("blob", (D, NBLOB))
    wg3 = din("wg3", (2 * O + 1, D))
    out = nc.declare_dram_parameter("out", [B, D, NPC], f32, isOutput=True)

    with tile.TileContext(nc) as tc:
        with (
            tc.tile_pool(name="const", bufs=1) as cp,
            tc.tile_pool(name="hp", bufs=4) as hp,
            tc.tile_pool(name="r9p", bufs=2) as r9p,
            tc.tile_pool(name="gsb", bufs=3) as gsb,
            tc.tile_pool(name="sm", bufs=2) as sm,
            tc.tile_pool(name="psA", bufs=4, space="PSUM") as psA,
        ):
            def ct(dram, shape, dt=f32, tag=None):
                t = cp.tile(list(shape), dt, tag=tag, name=tag)
                nc.sync.dma_start(t[:], dram[:])
                return t

            blob_t = ct(blob, (D, NBLOB), tag="blob")
            w1r_t = blob_t[:, 0 * D : 1 * D]
            w1s_t = blob_t[:, 1 * D : 2 * D]
            w2_f = blob_t[:, 2 * D : 3 * D]
            wu1x_t = blob_t[:, 3 * D : 4 * D]
            wu1a_t = blob_t[:, 4 * D : 5 * D]
            wu2_t = blob_t[:, 5 * D : 6 * D]
            cinv_t = blob_t[:, 6 * D : 6 * D + B * NPC]
            bofs = 6 * D + B * NPC
            b1c_t = blob_t[:, bofs + 0 : bofs + 1]
            bgc_t = blob_t[:, bofs + 1 : bofs + 2]
            b2c_t = blob_t[:, bofs + 2 : bofs + 3]
            bu1c_t = blob_t[:, bofs + 3 : bofs + 4]
            bu2c_t = blob_t[:, bofs + 4 : bofs + 5]

            # weights copied into each hot matmul's dtype
            if hdt != f32:
                w2t_ = cp.tile([D, D], hdt, tag="w2", name="w2")
                nc.vector.tensor_copy(w2t_[:], w2_f)
                w2_t = w2t_[:]
            else:
                w2_t = w2_f
            wg3_f = ct(wg3, (2 * O + 1, D), tag="wg3f")
            if rdt != f32:
                wg3t_ = cp.tile([2 * O + 1, D], rdt, tag="wg3", name="wg3")
                nc.vector.tensor_copy(wg3t_[:], wg3_f[:])
                wg3_t = wg3t_
            else:
                wg3_t = wg3_f

            xt_t, p8_t, p8r_t = [], [], []
            for b in range(B):
                xt_b = ct(xt[b], (D, N), tag=f"xt{b}")
                p8_b = ct(p8[b], (2 * O, N), rdt, tag=f"p8{b}")
                p8r_b = ct(p8r[b], (2 * O, NPC), tag=f"p8r{b}")
                xt_t.append(xt_b)
                p8_t.append(p8_b)
                p8r_t.append(p8r_b)
            xtr_all = ct(xtr, (D, B * NPC), tag="xtr")
            araw_all = cp.tile([D, B * NPC], f32, tag="araw", name="araw")
            zeros_t = cp.tile([D, N], sdt, tag="zeros", name="zeros")
            nc.vector.memset(zeros_t[:], 0.0)

            def relu_h(eng, H, src, bias_col):
                # H = max(src + bias_col, 0), one fused pass per engine
                if eng == "act":
                    nc.scalar.activation(H, src, AF.Relu, bias=bias_col)
                elif eng == "pool":
                    # full-128-partition op: all 8 Q7 cores engaged (the
                    # two-op tensor_scalar is the only relu form gpsimd
                    # compiles; STT fails walrus lower_dve)
                    nc.gpsimd.tensor_scalar(H, src, bias_col, 0.0,
                                            op0=A.add, op1=A.max)
                else:
                    nc.vector.scalar_tensor_tensor(
                        H, src, bias_col, zeros_t[:], op0=A.add, op1=A.max)

            def r9_rows(eng, dst, src, scal_col):
                if eng == "pool":
                    nc.gpsimd.tensor_scalar(dst, src, scal_col, None,
                                            op0=A.mult)
                elif eng == "act":
                    nc.scalar.activation(dst, src, AF.Copy, scale=scal_col)
                else:
                    nc.vector.tensor_scalar(dst, src, scal_col, None,
                                            op0=A.mult)

            # loop-invariant setup: send/recv projections (inputs are
            # constant across REPEAT iterations)
            sendT = []
            for b in range(B):
                # send projection: (D, N) = W1s.T @ XT (setup: plain fp32)
                s_ps = psA.tile([D, N], f32, tag="ps", name="s_ps")
                nc.tensor.matmul(s_ps[:], w1s_t, xt_t[b][:],
                                 start=True, stop=True)
                s_sb = cp.tile([D, N], sdt, tag=f"send{b}", name=f"send{b}")
                nc.scalar.copy(s_sb[:], s_ps[:])
                sendT.append(s_sb)
            # receiver projection + b1 for BOTH batches in one matmul
            r_ps = psA.tile([D, B * NPC], f32, tag="ps", name="r_ps")
            nc.tensor.matmul(r_ps[:], w1r_t, xtr_all[:],
                             start=True, stop=True)
            recvb_all = cp.tile([D, B * NPC], f32, tag="recvb",
                                name="recvb")
            nc.scalar.add(recvb_all[:], r_ps[:], b1c_t)

            loop_cm = (tc.For_i(0, REPEAT) if REPEAT > 1
                       else contextlib.nullcontext())
            with loop_cm:
              for iq in range(NPC // QUAD):
                i0 = iq * QUAD
                # mask rows are shared by both batches: one fp32r DMA per iq
                r94t = r9p.tile([2 * O + 1, QUAD * N], rdt, name="r94t",
                                bufs=2)
                r94 = r94t[:]
                nc.sync.dma_start(r94[2 * O : 2 * O + 1, :],
                                  mneg[iq : iq + 1, :])
                for b in range(B):
                    if COMBINE == "reduce16":
                        mg8 = hp.tile([D, QUAD * N], gdt, tag="mg8",
                                      name="mg8", bufs=2)
                    for g in range(QUAD // GRP):
                        m_ps = psA.tile([D, GRP * N], f32, tag="ps",
                                        name="m_ps")
                        g_ps = psA.tile([D, GRP * N], f32, tag="ps",
                                        name="g_ps")
                        gs = gsb.tile([D, GRP * N], gdt, name="gs")
                        for h in range(GRP):
                            i = g * GRP + h          # receiver within quad
                            col = b * NPC + i0 + i   # araw/recvb column
                            seg = slice(i * N, (i + 1) * N)
                            # gate rhs rows 0-7: phase products, per receiver
                            r9_rows(R9_ENG[i], r94[0 : 2 * O, seg], p8_t[b][:],
                                    p8r_t[b][:, i0 + i : i0 + i + 1])
                            # H = relu(send + recv_i + b1), fused one-pass
                            H = hp.tile([D, N], hdt, name="H", bufs=4)
                            relu_h(RELU_ENG[i], H[:], sendT[b][:],
                                   recvb_all[:, col : col + 1])
                            nc.tensor.matmul(m_ps[:, h * N : (h + 1) * N],
                                             w2_t, H[:], start=True, stop=True)
                            nc.tensor.matmul(g_ps[:, h * N : (h + 1) * N],
                                             wg3_t[:], r94[:, seg],
                                             start=True, stop=True)
                        # gate = sigmoid(glin + bg) for the group
                        nc.scalar.activation(gs[:], g_ps[:], AF.Sigmoid,
                                             bias=bgc_t)
                        # (messages + b2) * gate, with fused neighbor-sum
                        if COMBINE == "sttacc":
                            for h in range(GRP):
                                i = g * GRP + h
                                col = b * NPC + i0 + i
                                nc.vector.scalar_tensor_tensor(
                                    m_ps[:, h * N : (h + 1) * N],
                                    m_ps[:, h * N : (h + 1) * N], b2c_t,
                                    gs[:, h * N : (h + 1) * N],
                                    op0=A.add, op1=A.mult,
                                    accum_out=araw_all[:, col : col + 1])
                        else:
                            nc.vector.scalar_tensor_tensor(
                                mg8[:, g * GRP * N : (g + 1) * GRP * N],
                                m_ps[:], b2c_t, gs[:],
                                op0=A.add, op1=A.mult)
                    if COMBINE == "reduce16":
                        nc.vector.reduce_sum(
                            araw_all[:, b * NPC + i0 : b * NPC + i0 + QUAD],
                            mg8[:].rearrange("p (a b) -> p a b", a=QUAD),
                            axis=mybir.AxisListType.X)

              # update net + residual for BOTH batches in one set of ops
              aggt = sm.tile([D, B * NPC], f32, name="aggt")
              nc.vector.tensor_tensor(aggt[:], araw_all[:], cinv_t, A.mult)
              u_ps = psA.tile([D, B * NPC], f32, tag="ps", name="u_ps")
              nc.tensor.matmul(u_ps[:], wu1x_t, xtr_all[:],
                               start=True, stop=False)
              nc.tensor.matmul(u_ps[:], wu1a_t, aggt[:],
                               start=False, stop=True)
              hT = sm.tile([D, B * NPC], f32, name="hT")
              nc.scalar.activation(hT[:], u_ps[:], AF.Relu, bias=bu1c_t)
              o_ps = psA.tile([D, B * NPC], f32, tag="ps", name="o_ps")
              nc.tensor.matmul(o_ps[:], wu2_t, hT[:], start=True, stop=True)
              o_all = sm.tile([D, B * NPC], f32, name="o_all", bufs=1)
              nc.vector.scalar_tensor_tensor(
                  o_all[:], o_ps[:], bu2c_t, xtr_all[:],
                  op0=A.add, op1=A.add)
              nc.sync.dma_start(out[:].rearrange("b d n -> d b n"),
                                o_all[:].rearrange("d (b n) -> d b n", b=B))

    nc.compile()
    return nc


def _get_program():
    key = (H_DT, R9_DT, S_DT, G_DT, REPEAT, GRP, RELU_ENG, R9_ENG, COMBINE)
    if key not in _CACHE:
        _CACHE[key] = _build_program()
    return _CACHE[key]


class _Runner:
    """Compiled-executable cache around the bass2jax multi-core path.

    run_bass_kernel_spmd rebuilds the jit closure on every call, so each
    call pays a full retrace + BIR serialize + cache hash of the whole
    program (~30 us per instruction of host time).  Tracing once and
    reusing the compiled callable makes per-call cost transfer+execute.
    """

    def __init__(self, nc, n_cores):
        import jax
        from jax.experimental.shard_map import shard_map
        from jax.sharding import Mesh, PartitionSpec
        from concourse import bass2jax
        import concourse.mybir as mybir

        bass2jax.install_neuronx_cc_hook()
        self.n_cores = n_cores
        partition_name = (
            nc.partition_id_tensor.name if nc.partition_id_tensor else None
        )
        in_names, out_names, out_avals, zero_outs = [], [], [], []
        for alloc in nc.m.functions[0].allocations:
            if not isinstance(alloc, mybir.MemoryLocationSet):
                continue
            name = alloc.memorylocations[0].name
            if alloc.kind == "ExternalInput":
                if name != partition_name:
                    in_names.append(name)
            elif alloc.kind == "ExternalOutput":
                shape = tuple(alloc.tensor_shape)
                dtype = mybir.dt.np(alloc.dtype)
                out_names.append(name)
                out_avals.append(jax.core.ShapedArray(shape, dtype))
                zero_outs.append(np.zeros(shape, dtype))
        n_params = len(in_names)
        n_outs = len(out_names)
        self.in_names = list(in_names)
        self.out_names = list(out_names)
        self.out_shapes = [a.shape for a in out_avals]
        self.zero_outs = zero_outs
        in_names_ext = in_names + out_names
        if partition_name is not None:
            in_names_ext.append(partition_name)

        def _body(*args):
            operands = list(args)
            if partition_name is not None:
                operands.append(bass2jax.partition_id_tensor())
            outs = bass2jax._bass_exec_p.bind(
                *operands,
                out_avals=tuple(out_avals),
                in_names=tuple(in_names_ext),
                out_names=tuple(out_names),
                lowering_input_output_aliases=(),
                sim_require_finite=True,
                sim_require_nnan=True,
                nc=nc,
            )
            return tuple(outs)

        devices = jax.devices()[:n_cores]
        mesh = Mesh(np.asarray(devices), ("core",))
        in_specs = (PartitionSpec("core"),) * (n_params + n_outs)
        out_specs = (PartitionSpec("core"),) * n_outs
        self._fn = jax.jit(
            shard_map(_body, mesh=mesh, in_specs=in_specs,
                      out_specs=out_specs, check_rep=False),
            donate_argnums=tuple(range(n_params, n_params + n_outs)),
            keep_unused=True,
        )

    def __call__(self, in_maps):
        nco = self.n_cores
        concat_in = [
            np.concatenate([np.asarray(m[name]) for m in in_maps], axis=0)
            for name in self.in_names
        ]
        concat_zeros = [
            np.zeros((nco * z.shape[0], *z.shape[1:]), z.dtype)
            for z in self.zero_outs
        ]
        out_arrs = self._fn(*concat_in, *concat_zeros)
        return [
            {
                name: np.asarray(out_arrs[i]).reshape(nco, *self.out_shapes[i])[c]
                for i, name in enumerate(self.out_names)
            }
            for c in range(nco)
        ]


def _get_runner():
    key = ("runner", H_DT, R9_DT, S_DT, G_DT, REPEAT, GRP, RELU_ENG, R9_ENG, COMBINE)
    if key not in _CACHE:
        _CACHE[key] = _Runner(_get_program(), NCORES)
    return _CACHE[key]


def kernel(node_features, node_phases, adjacency,
           W1r, W1s, b1, W2, b2, Wg, bg, Wu1x, Wu1a, bu1, Wu2, bu2,
           _trace=False, _trace_kwargs=None):
    f4 = np.float32
    x = np.asarray(node_features, f4)
    ph = np.asarray(node_phases, f4)
    adj = np.asarray(adjacency)

    import concourse.mybir as mybir
    rnp = mybir.dt.np(getattr(mybir.dt, R9_DT))

    mask = (adj != 0)
    counts = np.maximum(mask.sum(axis=1), 1).astype(f4)          # (N,)
    cinv_full = (1.0 / counts)                                    # (N,)

    mneg_full = (MASK_NEG * (~mask)).astype(rnp)                  # (N, N)

    xt_full = np.ascontiguousarray(x.transpose(0, 2, 1))          # (B, D, N)
    p8_full = np.ascontiguousarray(
        np.concatenate([np.cos(ph), np.sin(ph)], axis=2).transpose(0, 2, 1))

    common = dict(
        xt=xt_full,
        p8=p8_full.astype(rnp),
        wg3=np.ascontiguousarray(
            np.concatenate([np.asarray(Wg, f4), np.asarray(Wg, f4),
                            np.ones((1, D), f4)], axis=0)),
    )

    in_maps = []
    for c in range(NCORES):
        lo, hi = c * NPC, (c + 1) * NPC
        m = dict(common)
        m["xtr"] = np.ascontiguousarray(
            np.concatenate([xt_full[b][:, lo:hi] for b in range(B)], axis=1))
        m["p8r"] = np.ascontiguousarray(p8_full[:, :, lo:hi])
        m["mneg"] = np.ascontiguousarray(
            mneg_full[lo:hi, :]).reshape(NPC // QUAD, QUAD * N)
        cinvb = np.broadcast_to(cinv_full[lo:hi][None, :], (D, NPC))
        m["blob"] = np.ascontiguousarray(np.concatenate(
            [np.asarray(W1r, f4), np.asarray(W1s, f4), np.asarray(W2, f4),
             np.asarray(Wu1x, f4), np.asarray(Wu1a, f4), np.asarray(Wu2, f4),
             cinvb, cinvb,
             np.asarray(b1, f4).reshape(D, 1), np.asarray(bg, f4).reshape(D, 1),
             np.asarray(b2, f4).reshape(D, 1), np.asarray(bu1, f4).reshape(D, 1),
             np.asarray(bu2, f4).reshape(D, 1)], axis=1))
        in_maps.append(m)

    results = _get_runner()(in_maps)

    out = np.empty((B, N, D), f4)
    for c in range(NCORES):
        lo, hi = c * NPC, (c + 1) * NPC
        out[:, lo:hi, :] = results[c]["out"].transpose(0, 2, 1)

    kernel.last_results = results
    return out

